# revision 31
# baseline (speedup 1.0000x reference)
"""Causal self-attention Trainium2 kernel, 8-core SPMD (token-sharded, collective-free).

Model: B=4, T=2048, D=1024, H=16 heads x 64. out = softmax(mask(QK^T/8)) V W_proj^T.

Sharding: 2 cores per batch. Core c handles batch c//2 and the 8 query tiles
(128 tokens each) at real positions t = 2j + (c%2), j=0..7 -- an interleaved
split so the causal work per core is balanced. Each core computes K/V for the
whole batch (modest recompute), attention for its own queries, and the output
projection for its own rows. No collectives; the causal structure difference
between even/odd cores is encoded purely in input data (mask tiles), so the
SPMD program is identical on all cores.

Schedule (v2): the PE never waits on the softmax chain --
  - pair p's attention ki-loop interleaves pair p+1's K/Q projection as 24
    2-matmul pieces (one per ki), so exp latency is hidden behind dense PE work
  - PV lags QK by one ki (software pipeline), so exp(ki) is always done
  - softmax normalize uses DVE reciprocal + gpsimd partition_broadcast
    (no DRAM round-trips), releasing the PV psum banks quickly
  - scalar engine runs ONLY exp; all psum->sbuf casts run on DVE
  - dummy warm-up matmuls keep the PE HAM clock warm during the input DMA
  - input DMAs issue from three queues (sync/scalar/gpsimd) in parallel

All matmuls run in bf16 (fp32 accumulate). Output is written bf16 (psum fp32
values rounded once); verified rel err ~4e-3 vs the fp32 reference.
"""

import os
from contextlib import ExitStack

import numpy as np
import ml_dtypes

import concourse.bass as bass
import concourse.mybir as mybir
import concourse.tile as tile
from concourse import bacc
from concourse.bass_utils import run_bass_kernel_spmd

BF16 = mybir.dt.bfloat16
F32 = mybir.dt.float32
EXP = mybir.ActivationFunctionType.Exp
RECIP = mybir.ActivationFunctionType.Reciprocal

B, T, D = 4, 2048, 1024
H, DH = 16, 64
NCORES = 8
QT = 8           # q-tiles of 128 per core
KT = 16          # k-tiles of 128 per batch
NPAIR = 8        # head pairs
NWARM = 34       # dummy warm-up matmuls (HAM warm + cover input DMA latency)

_cached = {}


def _build_program():
    nc = bacc.Bacc("TRN2", name="causal_attn")

    x_kvT = nc.dram_tensor("x_kvT", [D, T], BF16, kind="ExternalInput")
    x_qT = nc.dram_tensor("x_qT", [D, 1024], BF16, kind="ExternalInput")
    w_qT = nc.dram_tensor("w_qT", [D, D], BF16, kind="ExternalInput")
    w_kT = nc.dram_tensor("w_kT", [D, D], BF16, kind="ExternalInput")
    w_vT = nc.dram_tensor("w_vT", [D, D], BF16, kind="ExternalInput")
    w_pT = nc.dram_tensor("w_pT", [D, D], BF16, kind="ExternalInput")
    maskd = nc.dram_tensor("mask", [128, 256], BF16, kind="ExternalInput")
    outd = nc.dram_tensor("out_T", [D, 1024], BF16, kind="ExternalOutput")

    with ExitStack() as ctx:
        tc = ctx.enter_context(tile.TileContext(nc))

        # ---- pools ----
        const = ctx.enter_context(tc.tile_pool(name="const", bufs=1))
        vpool = ctx.enter_context(tc.tile_pool(name="vsb", bufs=1))
        opool = ctx.enter_context(tc.tile_pool(name="osb", bufs=1))
        kpool = ctx.enter_context(tc.tile_pool(name="ksb", bufs=2))
        qpool = ctx.enter_context(tc.tile_pool(name="qsb", bufs=2))
        ppool = ctx.enter_context(tc.tile_pool(name="pex", bufs=6))
        rpool = ctx.enter_context(tc.tile_pool(name="recip", bufs=4))
        bpool = ctx.enter_context(tc.tile_pool(name="bcast", bufs=2))
        spool = ctx.enter_context(tc.tile_pool(name="stage", bufs=2))
        drp = ctx.enter_context(tc.tile_pool(name="rscratch", bufs=4, space="DRAM"))
        outsb = ctx.enter_context(tc.tile_pool(name="outsb", bufs=2))
        wpp = ctx.enter_context(tc.tile_pool(name="wp", bufs=1))
        wp = [wpp.tile([128, D], BF16, tag=f"wp{d}", name=f"wp{d}") for d in range(8)]
        # PSUM: st 2x2 banks + pv 2 banks + mm 2 banks = 8
        st_ps = ctx.enter_context(tc.tile_pool(name="st_ps", bufs=2, space="PSUM"))
        pv_ps = ctx.enter_context(tc.tile_pool(name="pv_ps", bufs=2, space="PSUM"))
        mm_ps = ctx.enter_context(tc.tile_pool(name="mm_ps", bufs=2, space="PSUM"))

        # ---- warm-up: dense dummy matmuls from t=0 (no DMA dependency) ----
        wdum = const.tile([128, 128], BF16)
        rdum = const.tile([128, 512], BF16)
        nc.vector.memset(wdum[:, :], 0.0)
        nc.vector.memset(rdum[:, :], 0.0)
        for _ in range(NWARM):
            ps = mm_ps.tile([128, 512], F32, tag="ps", name="warm")
            nc.tensor.matmul(ps[:, :], lhsT=wdum[:, :], rhs=rdum[:, :],
                             start=True, stop=True)

        mask_sb = const.tile([128, 256], BF16)
        V_sb = [vpool.tile([128, H, DH + 1], BF16, tag=f"v{m}", name=f"v{m}") for m in range(KT)]
        O_sb = [opool.tile([128, 1024], BF16, tag=f"o{p}", name=f"o{p}") for p in range(NPAIR)]

        with ExitStack() as s1:
            xkvp = s1.enter_context(tc.tile_pool(name="xkv", bufs=1))
            xqp = s1.enter_context(tc.tile_pool(name="xq", bufs=1))
            wqp = s1.enter_context(tc.tile_pool(name="wq", bufs=1))
            wkp = s1.enter_context(tc.tile_pool(name="wk", bufs=1))
            wvp = s1.enter_context(tc.tile_pool(name="wv", bufs=1))
            xkv = [xkvp.tile([128, T], BF16, tag=f"xkv{d}", name=f"xkv{d}") for d in range(8)]
            xq = [xqp.tile([128, 1024], BF16, tag=f"xq{d}", name=f"xq{d}") for d in range(8)]
            wq = [wqp.tile([128, D], BF16, tag=f"wq{d}", name=f"wq{d}") for d in range(8)]
            wk = [wkp.tile([128, D], BF16, tag=f"wk{d}", name=f"wk{d}") for d in range(8)]
            wv = [wvp.tile([128, D], BF16, tag=f"wv{d}", name=f"wv{d}") for d in range(8)]

            # ---- input DMA: per-queue transfers serialize (~2.4us/256KB), so
            # round-robin every load across all three issue queues in exact
            # consumption order (V needs wv+cc0 first, prologue K/Q last)
            _q = [nc.sync, nc.scalar, nc.gpsimd]
            _qi = [0]

            def dq(out, in_):
                _q[_qi[0] % 3].dma_start(out=out, in_=in_)
                _qi[0] += 1

            nc.scalar.dma_start(out=mask_sb[:, :], in_=maskd[:, :])
            for d in range(8):
                dq(wv[d][:, :], w_vT[128 * d:128 * d + 128, :])
            for cc in range(4):
                for d in range(8):
                    dq(xkv[d][:, 512 * cc:512 * cc + 512],
                       x_kvT[128 * d:128 * d + 128, 512 * cc:512 * cc + 512])
            for d in range(8):
                dq(xq[d][:, :], x_qT[128 * d:128 * d + 128, :])
            for d in range(8):
                dq(wk[d][:, :], w_kT[128 * d:128 * d + 128, :])
            for d in range(8):
                dq(wq[d][:, :], w_qT[128 * d:128 * d + 128, :])
            for d in range(8):
                dq(wp[d][:, :], w_pT[128 * d:128 * d + 128, :])

            # ---- V projection (x stationary), strided into V_sb ----
            def emit_v(m):
                for n in range(2):
                    ps = mm_ps.tile([128, 512], F32, tag="ps", name="ps")
                    for d in range(8):
                        nc.tensor.matmul(
                            ps[:, :],
                            lhsT=xkv[d][:, 128 * m:128 * m + 128],
                            rhs=wv[d][:, 512 * n:512 * n + 512],
                            start=(d == 0), stop=(d == 7),
                        )
                    nc.vector.tensor_copy(
                        V_sb[m][:, 8 * n:8 * n + 8, 0:DH],
                        ps[:, :].rearrange("p (h e) -> p h e", h=8),
                    )
                nc.vector.memset(V_sb[m][:, :, DH:DH + 1], 1.0)

            for m in range(KT):
                emit_v(m)

            # ---- K/Q projection pieces: 6 chunks x 4 pieces of 2 matmuls ----
            def make_burst_ops(p, K_t, Q_t):
                ops = []
                for c in range(6):
                    holder = {}
                    for piece in range(4):
                        def op(c=c, piece=piece, holder=holder, K_t=K_t, Q_t=Q_t, p=p):
                            if piece == 0:
                                holder["ps"] = mm_ps.tile([128, 512], F32, tag="ps", name="ps")
                            ps = holder["ps"]
                            for d in (2 * piece, 2 * piece + 1):
                                if c < 4:
                                    nc.tensor.matmul(
                                        ps[:, :],
                                        lhsT=wk[d][:, 128 * p:128 * p + 128],
                                        rhs=xkv[d][:, 512 * c:512 * c + 512],
                                        start=(d == 0), stop=(d == 7),
                                    )
                                else:
                                    n = c - 4
                                    nc.tensor.matmul(
                                        ps[:, :],
                                        lhsT=wq[d][:, 128 * p:128 * p + 128],
                                        rhs=xq[d][:, 512 * n:512 * n + 512],
                                        start=(d == 0), stop=(d == 7),
                                    )
                            if piece == 3:
                                if c < 4:
                                    nc.vector.tensor_copy(K_t[:, 512 * c:512 * c + 512], ps[:, :])
                                else:
                                    n = c - 4
                                    nc.vector.tensor_copy(Q_t[:, 512 * n:512 * n + 512], ps[:, :])
                        ops.append(op)
                return ops

            # pair 0's K/Q emitted densely as the pipeline prologue
            KQ = {}
            KQ[0] = (kpool.tile([128, T], BF16, tag="k", name="k0"),
                     qpool.tile([128, 1024], BF16, tag="q", name="q0"))
            for op in make_burst_ops(0, KQ[0][0], KQ[0][1]):
                op()

            def qk(K_t, Q_t, h_off, ki, q0, qw, st_out):
                nc.tensor.matmul(
                    st_out,
                    lhsT=K_t[h_off:h_off + 64, 128 * ki:128 * ki + 128],
                    rhs=Q_t[h_off:h_off + 64, q0:q0 + qw],
                    start=True, stop=True,
                )

            # deferred softmax-normalize closures, drained inside later ki
            # loops (keeps the recip chain off the PE critical path)
            pending = []

            def make_norm(p, q0, stg, hi):
                # three stages, emitted >=1 ki apart, so every engine-queue op
                # has its inputs already resolved when it reaches the engine:
                # spread sums wide via DRAM, reciprocal on [128,4], broadcast
                # back across the dh partitions, then scale
                ctxd = {}

                # DRAM-hop DMAs split across two queues (gpsimd writes, sync
                # reads): both engines issue in-order, so keeping each queue's
                # ops dependency-free avoids serial wait compounding
                def n1():
                    rd = drp.tile([512], F32, name="rd")
                    nc.gpsimd.dma_start(out=rd[:], in_=stg[64:65, :])
                    rs = rpool.tile([128, 4], F32, tag="rs", name="rs")
                    nc.sync.dma_start(out=rs[:, :], in_=rd.rearrange("(p f) -> p f", p=128))
                    ctxd["rs"] = rs

                def n2():
                    rs2 = rpool.tile([128, 4], F32, tag="rs", name="rs2")
                    nc.vector.reciprocal(rs2[:, :], ctxd["rs"][:, :])
                    rd2 = drp.tile([512], F32, name="rd2")
                    nc.gpsimd.dma_start(out=rd2.rearrange("(p f) -> p f", p=128), in_=rs2[:, :])
                    bc = bpool.tile([64, 512], F32, tag="bc", name="bc")
                    nc.sync.dma_start(
                        out=bc[:, :],
                        in_=bass.AP(tensor=rd2.tensor, offset=rd2.offset,
                                    ap=[[0, 64]] + list(rd2.ap)),
                    )
                    ctxd["bc"] = bc

                def n3():
                    nc.vector.tensor_mul(
                        O_sb[p][64 * hi:64 * hi + 64, q0:q0 + 512],
                        stg[0:64, :], ctxd["bc"][:, :],
                    )

                return [n1, n2, n3]

            # one output-projection unit (m, n): 8 accumulating matmuls, a
            # psum->sbuf cast on the idle scalar engine, then the out DMA
            def emit_proj_unit(m, n, part):
                if part == 0:
                    proj_ps["ps"] = mm_ps.tile([128, 512], F32, tag="ps", name="ps")
                ps = proj_ps["ps"]
                for p2 in (range(4) if part == 0 else range(4, 8)):
                    nc.tensor.matmul(
                        ps[:, :],
                        lhsT=wp[p2][:, 128 * m:128 * m + 128],
                        rhs=O_sb[p2][:, 512 * n:512 * n + 512],
                        start=(p2 == 0), stop=(p2 == 7),
                    )
                if part == 1:
                    ob = outsb.tile([128, 512], BF16, tag="ob", name="ob")
                    nc.scalar.copy(ob[:, :], ps[:, :])
                    eng = (nc.sync, nc.scalar, nc.gpsimd)[m % 3]
                    eng.dma_start(
                        out=outd[128 * m:128 * m + 128, 512 * n:512 * n + 512],
                        in_=ob[:, :])

            proj_ps = {}
            NFILL = 3  # proj n=0 units pulled into pair 7's J=1 as PE filler

            for p in range(NPAIR):
                K_t, Q_t = KQ[p]
                if p + 1 < NPAIR:
                    KQ[p + 1] = (kpool.tile([128, T], BF16, tag="k", name=f"k{p+1}"),
                                 qpool.tile([128, 1024], BF16, tag="q", name=f"q{p+1}"))
                    burst = make_burst_ops(p + 1, KQ[p + 1][0], KQ[p + 1][1])
                else:
                    burst = []
                bi = 0

                for J in range(2):
                    q0 = 512 * J
                    nbulk = 8 * J
                    nki = nbulk + 8
                    pvs = {}
                    for hi in (0, 1):
                        pvs[hi] = pv_ps.tile([65, 512], F32, tag="pv", name="pv")

                    prevs = []
                    for ki in range(nki):
                        e = ki - nbulk
                        qc0 = 0 if e < 0 else 128 * (e // 2)
                        nw = 512 - qc0
                        st = st_ps.tile([128, 2, 512], F32, tag="st", name="st")
                        for hi, h_off in ((0, 0), (1, 64)):
                            qk(K_t, Q_t, h_off, ki, q0 + qc0, nw, st[:, hi, 0:nw])
                        pb = ppool.tile([128, 2, 512], BF16, tag="pb", name="pb")
                        nc.scalar.activation(pb[:, :, 0:nw], st[:, :, 0:nw], EXP)
                        if e >= 0:
                            m0 = 128 * (e & 1)
                            # multiplicative 0/1 causal mask on both heads at
                            # once; the mask operand repeats via a 0-stride dim
                            msrc = mask_sb[:, m0:m0 + 128]
                            mrep = bass.AP(tensor=msrc.tensor, offset=msrc.offset,
                                           ap=[list(msrc.ap[0]), [0, 2], [1, 128]])
                            nc.vector.tensor_mul(pb[:, :, 0:128], pb[:, :, 0:128], mrep)
                        # dense PE filler: next pair's K/Q piece (2 matmuls);
                        # for the last pair, early proj n=0 units instead
                        # (their p=7 operand -- O_sb[7] J=0 -- normalizes by
                        # mid-J1 via the deferred drains)
                        if bi < len(burst):
                            burst[bi]()
                            bi += 1
                        elif p == NPAIR - 1 and J == 1 and ki >= nki - 2 * NFILL:
                            fk = ki - (nki - 2 * NFILL)
                            emit_proj_unit(fk // 2, 0, fk % 2)
                        # drain one deferred normalize stage per ki
                        if pending:
                            pending.pop(0)()
                        # PV lags two kis: exp(prev) is long done AND the
                        # first PV of a J lands after the previous J's pv
                        # banks are released (no pool stall)
                        if len(prevs) == 2:
                            pe, pqc0, pnw, ppb = prevs.pop(0)
                            for hi in (0, 1):
                                nc.tensor.matmul(
                                    pvs[hi][:, pqc0:pqc0 + pnw],
                                    lhsT=V_sb[pe][:, 2 * p + hi, :],
                                    rhs=ppb[:, hi, 0:pnw],
                                    start=(pe == 0), stop=False,
                                )
                        prevs.append((ki, qc0, nw, pb))

                    for pe, pqc0, pnw, ppb in prevs:
                        for hi in (0, 1):
                            nc.tensor.matmul(
                                pvs[hi][:, pqc0:pqc0 + pnw],
                                lhsT=V_sb[pe][:, 2 * p + hi, :],
                                rhs=ppb[:, hi, 0:pnw],
                                start=(pe == 0), stop=(pe == nki - 1),
                            )

                    # stage pv out of PSUM immediately (frees the banks for
                    # the next J); normalize itself is deferred
                    # interleave the two heads' stages (n1h0,n1h1,n2h0,...) so
                    # each norm's consecutive stages drain >=2 kis apart --
                    # their DMA results are resolved before the engine op runs
                    # stage copies split across scalar/vector so both pv
                    # banks release in parallel (~0.7us sooner)
                    stgs = {}
                    for hi in (0, 1):
                        stgs[hi] = spool.tile([65, 512], F32, tag="stg", name="stg")
                        if hi == 0:
                            nc.scalar.copy(stgs[hi][:, :], pvs[hi][:, :])
                        else:
                            nc.vector.tensor_copy(stgs[hi][:, :], pvs[hi][:, :])
                    norms = [make_norm(p, q0, stgs[hi], hi) for hi in (0, 1)]
                    if p == NPAIR - 1 and J == 1:
                        # final J: nothing left to overlap with -- emit the
                        # whole chain now (split queues keep it ~3.5us) so the
                        # projection's p=7 matmuls unblock quickly
                        for si in range(3):
                            for hi in (0, 1):
                                norms[hi][si]()
                    else:
                        for si in range(3):
                            for hi in (0, 1):
                                pending.append(norms[hi][si])

            # drain any remaining normalizes before the output projection
            while pending:
                pending.pop(0)()

        # ---- output projection (bf16 out) ----
        # n=0 units read only J=0 outputs (normalized long ago); sweeping them
        # first overlaps the final pair's J=1 normalize chain with matmuls and
        # keeps the PE warm. The first NFILL n=0 units already ran as pair-7
        # filler above.
        for n in range(2):
            for m in range(NFILL if n == 0 else 0, 8):
                emit_proj_unit(m, n, 0)
                emit_proj_unit(m, n, 1)

    nc.finalize()
    return nc


def _host_inputs(x, W_qkv, W_proj):
    bf = ml_dtypes.bfloat16
    wq = np.ascontiguousarray((W_qkv[0:D] / 8.0).T.astype(bf))
    wk = np.ascontiguousarray(W_qkv[D:2 * D].T.astype(bf))
    wv = np.ascontiguousarray(W_qkv[2 * D:3 * D].T.astype(bf))
    wp = np.ascontiguousarray(W_proj.T.astype(bf))

    kk, qq = np.meshgrid(np.arange(128), np.arange(128), indexing="ij")
    stair = (kk <= qq).astype(np.float32)
    masks = {
        0: np.concatenate([stair, np.zeros((128, 128), np.float32)], axis=1).astype(bf),
        1: np.concatenate([np.ones((128, 128), np.float32), stair], axis=1).astype(bf),
    }

    in_maps = []
    for c in range(NCORES):
        b, fold = c // 2, c % 2
        xT = np.ascontiguousarray(x[b].T.astype(bf))  # [D, T]
        qidx = np.concatenate(
            [np.arange(128 * (2 * j + fold), 128 * (2 * j + fold) + 128) for j in range(QT)]
        )
        in_maps.append({
            "x_kvT": xT,
            "x_qT": np.ascontiguousarray(xT[:, qidx]),
            "w_qT": wq, "w_kT": wk, "w_vT": wv, "w_pT": wp,
            "mask": np.ascontiguousarray(masks[fold]),
        })
    return in_maps


def _run(inputs, trace=False, trace_cores=None):
    if "nc" not in _cached:
        _cached["nc"] = _build_program()
    nc = _cached["nc"]
    in_maps = _host_inputs(inputs["x"], inputs["W_qkv"], inputs["W_proj"])
    res = run_bass_kernel_spmd(
        nc, in_maps, core_ids=list(range(NCORES)),
        trace=trace, trace_cores=trace_cores,
    )
    out = np.zeros((B, T, D), np.float32)
    for c in range(NCORES):
        b, fold = c // 2, c % 2
        oT = res.results[c]["out_T"].astype(np.float32)  # [D, 1024]
        for j in range(QT):
            t0 = 128 * (2 * j + fold)
            out[b, t0:t0 + 128, :] = oT[:, 128 * j:128 * j + 128].T
    return out, res


def kernel(**inputs) -> np.ndarray:
    out, _ = _run(inputs, trace=os.environ.get("KERNEL_TRACE", "") == "1")
    return out


# revision 32
# speedup vs baseline: 1.0007x; 1.0007x over previous
"""Causal self-attention Trainium2 kernel, 8-core SPMD (token-sharded, collective-free).

Model: B=4, T=2048, D=1024, H=16 heads x 64. out = softmax(mask(QK^T/8)) V W_proj^T.

Sharding: 2 cores per batch. Core c handles batch c//2 and the 8 query tiles
(128 tokens each) at real positions t = 2j + (c%2), j=0..7 -- an interleaved
split so the causal work per core is balanced. Each core computes K/V for the
whole batch (modest recompute), attention for its own queries, and the output
projection for its own rows. No collectives; the causal structure difference
between even/odd cores is encoded purely in input data (mask tiles), so the
SPMD program is identical on all cores.

Schedule (v2): the PE never waits on the softmax chain --
  - pair p's attention ki-loop interleaves pair p+1's K/Q projection as 24
    2-matmul pieces (one per ki), so exp latency is hidden behind dense PE work
  - PV lags QK by one ki (software pipeline), so exp(ki) is always done
  - softmax normalize uses DVE reciprocal + gpsimd partition_broadcast
    (no DRAM round-trips), releasing the PV psum banks quickly
  - scalar engine runs ONLY exp; all psum->sbuf casts run on DVE
  - dummy warm-up matmuls keep the PE HAM clock warm during the input DMA
  - input DMAs issue from three queues (sync/scalar/gpsimd) in parallel

All matmuls run in bf16 (fp32 accumulate). Output is written bf16 (psum fp32
values rounded once); verified rel err ~4e-3 vs the fp32 reference.
"""

import os
from contextlib import ExitStack

import numpy as np
import ml_dtypes

import concourse.bass as bass
import concourse.mybir as mybir
import concourse.tile as tile
from concourse import bacc
from concourse.bass_utils import run_bass_kernel_spmd

BF16 = mybir.dt.bfloat16
F32 = mybir.dt.float32
EXP = mybir.ActivationFunctionType.Exp
RECIP = mybir.ActivationFunctionType.Reciprocal

B, T, D = 4, 2048, 1024
H, DH = 16, 64
NCORES = 8
QT = 8           # q-tiles of 128 per core
KT = 16          # k-tiles of 128 per batch
NPAIR = 8        # head pairs
NWARM = 40       # dummy warm-up matmuls (HAM warm + cover input DMA latency)

_cached = {}


def _build_program():
    nc = bacc.Bacc("TRN2", name="causal_attn")

    x_kvT = nc.dram_tensor("x_kvT", [D, T], BF16, kind="ExternalInput")
    x_qT = nc.dram_tensor("x_qT", [D, 1024], BF16, kind="ExternalInput")
    w_qT = nc.dram_tensor("w_qT", [D, D], BF16, kind="ExternalInput")
    w_kT = nc.dram_tensor("w_kT", [D, D], BF16, kind="ExternalInput")
    w_vT = nc.dram_tensor("w_vT", [D, D], BF16, kind="ExternalInput")
    w_pT = nc.dram_tensor("w_pT", [D, D], BF16, kind="ExternalInput")
    maskd = nc.dram_tensor("mask", [128, 256], BF16, kind="ExternalInput")
    outd = nc.dram_tensor("out_T", [D, 1024], BF16, kind="ExternalOutput")

    with ExitStack() as ctx:
        tc = ctx.enter_context(tile.TileContext(nc))

        # ---- pools ----
        const = ctx.enter_context(tc.tile_pool(name="const", bufs=1))
        vpool = ctx.enter_context(tc.tile_pool(name="vsb", bufs=1))
        opool = ctx.enter_context(tc.tile_pool(name="osb", bufs=1))
        kpool = ctx.enter_context(tc.tile_pool(name="ksb", bufs=2))
        qpool = ctx.enter_context(tc.tile_pool(name="qsb", bufs=2))
        ppool = ctx.enter_context(tc.tile_pool(name="pex", bufs=6))
        rpool = ctx.enter_context(tc.tile_pool(name="recip", bufs=4))
        bpool = ctx.enter_context(tc.tile_pool(name="bcast", bufs=2))
        spool = ctx.enter_context(tc.tile_pool(name="stage", bufs=2))
        drp = ctx.enter_context(tc.tile_pool(name="rscratch", bufs=4, space="DRAM"))
        outsb = ctx.enter_context(tc.tile_pool(name="outsb", bufs=2))
        wpp = ctx.enter_context(tc.tile_pool(name="wp", bufs=1))
        wp = [wpp.tile([128, D], BF16, tag=f"wp{d}", name=f"wp{d}") for d in range(8)]
        # PSUM: st 2x2 banks + pv 2 banks + mm 2 banks = 8
        st_ps = ctx.enter_context(tc.tile_pool(name="st_ps", bufs=2, space="PSUM"))
        pv_ps = ctx.enter_context(tc.tile_pool(name="pv_ps", bufs=2, space="PSUM"))
        mm_ps = ctx.enter_context(tc.tile_pool(name="mm_ps", bufs=2, space="PSUM"))

        # ---- warm-up: dense dummy matmuls from t=0 (no DMA dependency) ----
        wdum = const.tile([128, 128], BF16)
        rdum = const.tile([128, 512], BF16)
        nc.vector.memset(wdum[:, :], 0.0)
        nc.vector.memset(rdum[:, :], 0.0)
        for _ in range(NWARM):
            ps = mm_ps.tile([128, 512], F32, tag="ps", name="warm")
            nc.tensor.matmul(ps[:, :], lhsT=wdum[:, :], rhs=rdum[:, :],
                             start=True, stop=True)

        mask_sb = const.tile([128, 256], BF16)
        V_sb = [vpool.tile([128, H, DH + 1], BF16, tag=f"v{m}", name=f"v{m}") for m in range(KT)]
        O_sb = [opool.tile([128, 1024], BF16, tag=f"o{p}", name=f"o{p}") for p in range(NPAIR)]

        with ExitStack() as s1:
            xkvp = s1.enter_context(tc.tile_pool(name="xkv", bufs=1))
            xqp = s1.enter_context(tc.tile_pool(name="xq", bufs=1))
            wqp = s1.enter_context(tc.tile_pool(name="wq", bufs=1))
            wkp = s1.enter_context(tc.tile_pool(name="wk", bufs=1))
            wvp = s1.enter_context(tc.tile_pool(name="wv", bufs=1))
            xkv = [xkvp.tile([128, T], BF16, tag=f"xkv{d}", name=f"xkv{d}") for d in range(8)]
            xq = [xqp.tile([128, 1024], BF16, tag=f"xq{d}", name=f"xq{d}") for d in range(8)]
            wq = [wqp.tile([128, D], BF16, tag=f"wq{d}", name=f"wq{d}") for d in range(8)]
            wk = [wkp.tile([128, D], BF16, tag=f"wk{d}", name=f"wk{d}") for d in range(8)]
            wv = [wvp.tile([128, D], BF16, tag=f"wv{d}", name=f"wv{d}") for d in range(8)]

            # ---- input DMA: per-queue transfers serialize (~2.4us/256KB), so
            # round-robin every load across all three issue queues in exact
            # consumption order (V needs wv+cc0 first, prologue K/Q last)
            _q = [nc.sync, nc.scalar, nc.gpsimd]
            _qi = [0]

            def dq(out, in_):
                _q[_qi[0] % 3].dma_start(out=out, in_=in_)
                _qi[0] += 1

            nc.scalar.dma_start(out=mask_sb[:, :], in_=maskd[:, :])
            for d in range(8):
                dq(wv[d][:, :], w_vT[128 * d:128 * d + 128, :])
            for cc in range(4):
                for d in range(8):
                    dq(xkv[d][:, 512 * cc:512 * cc + 512],
                       x_kvT[128 * d:128 * d + 128, 512 * cc:512 * cc + 512])
            for d in range(8):
                dq(xq[d][:, :], x_qT[128 * d:128 * d + 128, :])
            for d in range(8):
                dq(wk[d][:, :], w_kT[128 * d:128 * d + 128, :])
            for d in range(8):
                dq(wq[d][:, :], w_qT[128 * d:128 * d + 128, :])
            for d in range(8):
                dq(wp[d][:, :], w_pT[128 * d:128 * d + 128, :])

            # ---- V projection (x stationary), strided into V_sb ----
            def emit_v(m):
                for n in range(2):
                    ps = mm_ps.tile([128, 512], F32, tag="ps", name="ps")
                    for d in range(8):
                        nc.tensor.matmul(
                            ps[:, :],
                            lhsT=xkv[d][:, 128 * m:128 * m + 128],
                            rhs=wv[d][:, 512 * n:512 * n + 512],
                            start=(d == 0), stop=(d == 7),
                        )
                    nc.vector.tensor_copy(
                        V_sb[m][:, 8 * n:8 * n + 8, 0:DH],
                        ps[:, :].rearrange("p (h e) -> p h e", h=8),
                    )
                nc.vector.memset(V_sb[m][:, :, DH:DH + 1], 1.0)

            for m in range(KT):
                emit_v(m)

            # ---- K/Q projection pieces: 6 chunks x 4 pieces of 2 matmuls ----
            def make_burst_ops(p, K_t, Q_t):
                ops = []
                for c in range(6):
                    holder = {}
                    for piece in range(4):
                        def op(c=c, piece=piece, holder=holder, K_t=K_t, Q_t=Q_t, p=p):
                            if piece == 0:
                                holder["ps"] = mm_ps.tile([128, 512], F32, tag="ps", name="ps")
                            ps = holder["ps"]
                            for d in (2 * piece, 2 * piece + 1):
                                if c < 4:
                                    nc.tensor.matmul(
                                        ps[:, :],
                                        lhsT=wk[d][:, 128 * p:128 * p + 128],
                                        rhs=xkv[d][:, 512 * c:512 * c + 512],
                                        start=(d == 0), stop=(d == 7),
                                    )
                                else:
                                    n = c - 4
                                    nc.tensor.matmul(
                                        ps[:, :],
                                        lhsT=wq[d][:, 128 * p:128 * p + 128],
                                        rhs=xq[d][:, 512 * n:512 * n + 512],
                                        start=(d == 0), stop=(d == 7),
                                    )
                            if piece == 3:
                                if c < 4:
                                    nc.vector.tensor_copy(K_t[:, 512 * c:512 * c + 512], ps[:, :])
                                else:
                                    n = c - 4
                                    nc.vector.tensor_copy(Q_t[:, 512 * n:512 * n + 512], ps[:, :])
                        ops.append(op)
                return ops

            # pair 0's K/Q emitted densely as the pipeline prologue
            KQ = {}
            KQ[0] = (kpool.tile([128, T], BF16, tag="k", name="k0"),
                     qpool.tile([128, 1024], BF16, tag="q", name="q0"))
            for op in make_burst_ops(0, KQ[0][0], KQ[0][1]):
                op()

            def qk(K_t, Q_t, h_off, ki, q0, qw, st_out):
                nc.tensor.matmul(
                    st_out,
                    lhsT=K_t[h_off:h_off + 64, 128 * ki:128 * ki + 128],
                    rhs=Q_t[h_off:h_off + 64, q0:q0 + qw],
                    start=True, stop=True,
                )

            # deferred softmax-normalize closures, drained inside later ki
            # loops (keeps the recip chain off the PE critical path)
            pending = []

            def make_norm(p, q0, stg, hi):
                # three stages, emitted >=1 ki apart, so every engine-queue op
                # has its inputs already resolved when it reaches the engine:
                # spread sums wide via DRAM, reciprocal on [128,4], broadcast
                # back across the dh partitions, then scale
                ctxd = {}

                # DRAM-hop DMAs split across two queues (gpsimd writes, sync
                # reads): both engines issue in-order, so keeping each queue's
                # ops dependency-free avoids serial wait compounding
                def n1():
                    rd = drp.tile([512], F32, name="rd")
                    nc.gpsimd.dma_start(out=rd[:], in_=stg[64:65, :])
                    rs = rpool.tile([128, 4], F32, tag="rs", name="rs")
                    nc.sync.dma_start(out=rs[:, :], in_=rd.rearrange("(p f) -> p f", p=128))
                    ctxd["rs"] = rs

                def n2():
                    rs2 = rpool.tile([128, 4], F32, tag="rs", name="rs2")
                    nc.vector.reciprocal(rs2[:, :], ctxd["rs"][:, :])
                    rd2 = drp.tile([512], F32, name="rd2")
                    nc.gpsimd.dma_start(out=rd2.rearrange("(p f) -> p f", p=128), in_=rs2[:, :])
                    bc = bpool.tile([64, 512], F32, tag="bc", name="bc")
                    nc.sync.dma_start(
                        out=bc[:, :],
                        in_=bass.AP(tensor=rd2.tensor, offset=rd2.offset,
                                    ap=[[0, 64]] + list(rd2.ap)),
                    )
                    ctxd["bc"] = bc

                def n3():
                    nc.vector.tensor_mul(
                        O_sb[p][64 * hi:64 * hi + 64, q0:q0 + 512],
                        stg[0:64, :], ctxd["bc"][:, :],
                    )

                return [n1, n2, n3]

            # one output-projection unit (m, n): 8 accumulating matmuls, a
            # psum->sbuf cast on the idle scalar engine, then the out DMA
            def emit_proj_unit(m, n, part):
                if part == 0:
                    proj_ps["ps"] = mm_ps.tile([128, 512], F32, tag="ps", name="ps")
                ps = proj_ps["ps"]
                for p2 in (range(4) if part == 0 else range(4, 8)):
                    nc.tensor.matmul(
                        ps[:, :],
                        lhsT=wp[p2][:, 128 * m:128 * m + 128],
                        rhs=O_sb[p2][:, 512 * n:512 * n + 512],
                        start=(p2 == 0), stop=(p2 == 7),
                    )
                if part == 1:
                    ob = outsb.tile([128, 512], BF16, tag="ob", name="ob")
                    nc.scalar.copy(ob[:, :], ps[:, :])
                    eng = (nc.sync, nc.scalar, nc.gpsimd)[m % 3]
                    eng.dma_start(
                        out=outd[128 * m:128 * m + 128, 512 * n:512 * n + 512],
                        in_=ob[:, :])

            proj_ps = {}
            NFILL = 3  # proj n=0 units pulled into pair 7's J=1 as PE filler

            for p in range(NPAIR):
                K_t, Q_t = KQ[p]
                if p + 1 < NPAIR:
                    KQ[p + 1] = (kpool.tile([128, T], BF16, tag="k", name=f"k{p+1}"),
                                 qpool.tile([128, 1024], BF16, tag="q", name=f"q{p+1}"))
                    burst = make_burst_ops(p + 1, KQ[p + 1][0], KQ[p + 1][1])
                else:
                    burst = []
                bi = 0

                for J in range(2):
                    q0 = 512 * J
                    nbulk = 8 * J
                    nki = nbulk + 8
                    pvs = {}
                    for hi in (0, 1):
                        pvs[hi] = pv_ps.tile([65, 512], F32, tag="pv", name="pv")

                    prevs = []
                    for ki in range(nki):
                        e = ki - nbulk
                        qc0 = 0 if e < 0 else 128 * (e // 2)
                        nw = 512 - qc0
                        st = st_ps.tile([128, 2, 512], F32, tag="st", name="st")
                        for hi, h_off in ((0, 0), (1, 64)):
                            qk(K_t, Q_t, h_off, ki, q0 + qc0, nw, st[:, hi, 0:nw])
                        pb = ppool.tile([128, 2, 512], BF16, tag="pb", name="pb")
                        nc.scalar.activation(pb[:, :, 0:nw], st[:, :, 0:nw], EXP)
                        if e >= 0:
                            m0 = 128 * (e & 1)
                            # multiplicative 0/1 causal mask on both heads at
                            # once; the mask operand repeats via a 0-stride dim
                            msrc = mask_sb[:, m0:m0 + 128]
                            mrep = bass.AP(tensor=msrc.tensor, offset=msrc.offset,
                                           ap=[list(msrc.ap[0]), [0, 2], [1, 128]])
                            nc.vector.tensor_mul(pb[:, :, 0:128], pb[:, :, 0:128], mrep)
                        # dense PE filler: next pair's K/Q piece (2 matmuls);
                        # for the last pair, early proj n=0 units instead
                        # (their p=7 operand -- O_sb[7] J=0 -- normalizes by
                        # mid-J1 via the deferred drains)
                        if bi < len(burst):
                            burst[bi]()
                            bi += 1
                        elif p == NPAIR - 1 and J == 1 and ki >= nki - 2 * NFILL:
                            fk = ki - (nki - 2 * NFILL)
                            emit_proj_unit(fk // 2, 0, fk % 2)
                        # drain one deferred normalize stage per ki
                        if pending:
                            pending.pop(0)()
                        # PV lags two kis: exp(prev) is long done AND the
                        # first PV of a J lands after the previous J's pv
                        # banks are released (no pool stall)
                        if len(prevs) == 2:
                            pe, pqc0, pnw, ppb = prevs.pop(0)
                            for hi in (0, 1):
                                nc.tensor.matmul(
                                    pvs[hi][:, pqc0:pqc0 + pnw],
                                    lhsT=V_sb[pe][:, 2 * p + hi, :],
                                    rhs=ppb[:, hi, 0:pnw],
                                    start=(pe == 0), stop=False,
                                )
                        prevs.append((ki, qc0, nw, pb))

                    for pe, pqc0, pnw, ppb in prevs:
                        for hi in (0, 1):
                            nc.tensor.matmul(
                                pvs[hi][:, pqc0:pqc0 + pnw],
                                lhsT=V_sb[pe][:, 2 * p + hi, :],
                                rhs=ppb[:, hi, 0:pnw],
                                start=(pe == 0), stop=(pe == nki - 1),
                            )

                    # stage pv out of PSUM immediately (frees the banks for
                    # the next J); normalize itself is deferred
                    # interleave the two heads' stages (n1h0,n1h1,n2h0,...) so
                    # each norm's consecutive stages drain >=2 kis apart --
                    # their DMA results are resolved before the engine op runs
                    # stage copies split across scalar/vector so both pv
                    # banks release in parallel (~0.7us sooner)
                    stgs = {}
                    for hi in (0, 1):
                        stgs[hi] = spool.tile([65, 512], F32, tag="stg", name="stg")
                        if hi == 0:
                            nc.scalar.copy(stgs[hi][:, :], pvs[hi][:, :])
                        else:
                            nc.vector.tensor_copy(stgs[hi][:, :], pvs[hi][:, :])
                    norms = [make_norm(p, q0, stgs[hi], hi) for hi in (0, 1)]
                    if p == NPAIR - 1 and J == 1:
                        # final J: nothing left to overlap with -- emit the
                        # whole chain now (split queues keep it ~3.5us) so the
                        # projection's p=7 matmuls unblock quickly
                        for si in range(3):
                            for hi in (0, 1):
                                norms[hi][si]()
                    else:
                        for si in range(3):
                            for hi in (0, 1):
                                pending.append(norms[hi][si])

            # drain any remaining normalizes before the output projection
            while pending:
                pending.pop(0)()

        # ---- output projection (bf16 out) ----
        # n=0 units read only J=0 outputs (normalized long ago); sweeping them
        # first overlaps the final pair's J=1 normalize chain with matmuls and
        # keeps the PE warm. The first NFILL n=0 units already ran as pair-7
        # filler above.
        for n in range(2):
            for m in range(NFILL if n == 0 else 0, 8):
                emit_proj_unit(m, n, 0)
                emit_proj_unit(m, n, 1)

    nc.finalize()
    return nc


def _host_inputs(x, W_qkv, W_proj):
    bf = ml_dtypes.bfloat16
    wq = np.ascontiguousarray((W_qkv[0:D] / 8.0).T.astype(bf))
    wk = np.ascontiguousarray(W_qkv[D:2 * D].T.astype(bf))
    wv = np.ascontiguousarray(W_qkv[2 * D:3 * D].T.astype(bf))
    wp = np.ascontiguousarray(W_proj.T.astype(bf))

    kk, qq = np.meshgrid(np.arange(128), np.arange(128), indexing="ij")
    stair = (kk <= qq).astype(np.float32)
    masks = {
        0: np.concatenate([stair, np.zeros((128, 128), np.float32)], axis=1).astype(bf),
        1: np.concatenate([np.ones((128, 128), np.float32), stair], axis=1).astype(bf),
    }

    in_maps = []
    for c in range(NCORES):
        b, fold = c // 2, c % 2
        xT = np.ascontiguousarray(x[b].T.astype(bf))  # [D, T]
        qidx = np.concatenate(
            [np.arange(128 * (2 * j + fold), 128 * (2 * j + fold) + 128) for j in range(QT)]
        )
        in_maps.append({
            "x_kvT": xT,
            "x_qT": np.ascontiguousarray(xT[:, qidx]),
            "w_qT": wq, "w_kT": wk, "w_vT": wv, "w_pT": wp,
            "mask": np.ascontiguousarray(masks[fold]),
        })
    return in_maps


def _run(inputs, trace=False, trace_cores=None):
    if "nc" not in _cached:
        _cached["nc"] = _build_program()
    nc = _cached["nc"]
    in_maps = _host_inputs(inputs["x"], inputs["W_qkv"], inputs["W_proj"])
    res = run_bass_kernel_spmd(
        nc, in_maps, core_ids=list(range(NCORES)),
        trace=trace, trace_cores=trace_cores,
    )
    out = np.zeros((B, T, D), np.float32)
    for c in range(NCORES):
        b, fold = c // 2, c % 2
        oT = res.results[c]["out_T"].astype(np.float32)  # [D, 1024]
        for j in range(QT):
            t0 = 128 * (2 * j + fold)
            out[b, t0:t0 + 128, :] = oT[:, 128 * j:128 * j + 128].T
    return out, res


def kernel(**inputs) -> np.ndarray:
    out, _ = _run(inputs, trace=os.environ.get("KERNEL_TRACE", "") == "1")
    return out
